# revision 16
# baseline (speedup 1.0000x reference)
"""MoE-routed low-rank attention (nn_NeuronCircuit_28930899706061) on 8 TRN2 cores.

Sharding: core c in 0..7 -> batch b = c//4, token-shard q4 = c%4 (512 tokens)
for the compress/routing phase; head group heads [4*q4, 4*q4+4) of batch b for
the attention phase. h^T tensors are all-gathered within each 4-core batch
group; each core emits a partial [S, D] output (its 4 heads' contribution
through wO) and the host sums the 4 partials per batch.

All weight transposes are done host-side so every device DMA is contiguous.
"""

import numpy as np

B, S, D, H, RANK, NCMP = 2, 2048, 1024, 16, 128, 16
DH = D // H  # 64
N_CORES = 8
SHARD = S // 4  # 512 tokens per core in phase 1
HPC = 4  # heads per core
QT_TILES = S // 128  # 16 q tiles
CH = 512  # k-chunk width (one PSUM bank)
NCHUNK = S // CH  # 4

_RUNNERS: dict = {}


def _split_multi_waits(nc, mybir):
    """This toolchain's walrus rejects any instruction carrying >1 sync wait
    ("Too many sync wait commands"); hoist excess waits onto same-engine nops
    inserted immediately before the instruction."""
    cnt = 0
    for f in nc.m.functions:
        for blk in f.blocks:
            il = blk.instructions
            out = []
            changed = False
            for inst in il:
                si = inst.sync_info
                waits = list(si.on_wait or []) if si else []
                if len(waits) > 1:
                    for w in waits[:-1]:
                        cnt += 1
                        nop = mybir.InstNoOp(
                            name=f"wsplit-{cnt}",
                            engine=inst.engine,
                            sync_info=mybir.SyncInfo(on_wait=[w], on_update=[]),
                        )
                        nc.register_instruction(nop)
                        out.append(nop)
                    inst.sync_info = mybir.SyncInfo(
                        on_wait=[waits[-1]], on_update=list(si.on_update or [])
                    )
                    changed = True
                out.append(inst)
            if changed:
                il[:] = out


def _make_tc_class(tile, mybir):
    class TC(tile.TileContext):
        def __exit__(self, *exc):
            ret = super().__exit__(*exc)
            if exc[0] is None:
                _split_multi_waits(self.nc, mybir)
            return ret

    return TC


def _mask_plan(maskb):
    """Per q-tile chunk statuses + true k-tile count, plus packed mask tiles.

    Returns (plan, tiles) where plan[qi] = (chunk_stats, nch, nkt) with
    chunk_stats a list of ('full'|'off'|int mask-tile index) for chunks
    0..nch-1, and tiles a [nt,128,512] f32 array of additive masks.
    """
    tiles = []
    tile_ids = {}
    plan = []
    for qi in range(QT_TILES):
        rows = maskb[qi * 128 : (qi + 1) * 128]
        stats = []
        for c in range(NCHUNK):
            sub = rows[:, c * CH : (c + 1) * CH]
            if sub.all():
                stats.append("full")
            elif not sub.any():
                stats.append("off")
            else:
                add = np.where(sub, np.float32(0), np.float32(-1e30))
                key = add.tobytes()
                if key not in tile_ids:
                    tile_ids[key] = len(tiles)
                    tiles.append(add)
                stats.append(tile_ids[key])
        # trim trailing 'off' chunks
        nch = NCHUNK
        while nch > 0 and stats[nch - 1] == "off":
            nch -= 1
        # true k-tiles (128 wide) with any active column
        nkt = 0
        for kt in range(S // 128):
            if rows[:, kt * 128 : (kt + 1) * 128].any():
                nkt = kt + 1
        plan.append((tuple(stats[:nch]), nch, nkt))
    nt = len(tiles)
    tiles_arr = (
        np.stack(tiles).astype(np.float32) if nt else np.zeros((0, 128, CH), np.float32)
    )
    return tuple(plan), tiles_arr


def _bcast_mid(bass, ap, n):
    """[P, K] AP -> [P, n, K] AP with a step-0 middle dim (free broadcast)."""
    dims = [list(x) for x in ap.ap]
    return bass.AP(
        tensor=ap.tensor, offset=ap.offset, ap=[dims[0], [0, n]] + dims[1:]
    )


def _build(plan, nt):
    import concourse.bass as bass
    import concourse.mybir as mybir
    import concourse.tile as tile
    from concourse.bass import ts
    from concourse.masks import make_identity

    f32 = mybir.dt.float32
    Exp = mybir.ActivationFunctionType.Exp
    TC = _make_tc_class(tile, mybir)

    nc = bass.Bass(num_devices=N_CORES)
    xT_d = nc.dram_tensor("xT", [8, 128, SHARD], f32, kind="ExternalInput")
    cflat_d = nc.dram_tensor("cflat", [8, 128, RANK * NCMP], f32, kind="ExternalInput")
    routersT_d = nc.dram_tensor("routersT", [8, 128, 48], f32, kind="ExternalInput")
    wqkvT_d = nc.dram_tensor("wqkvT", [3, 128, HPC * DH], f32, kind="ExternalInput")
    wOT_d = nc.dram_tensor("wOT", [2, 128, D], f32, kind="ExternalInput")
    dmask_d = (
        nc.dram_tensor("dmask", [nt, 128, CH], f32, kind="ExternalInput")
        if nt
        else None
    )
    out_d = nc.dram_tensor("out", [S, D], f32, kind="ExternalOutput")

    with TC(nc) as tc:
        with (
            tc.tile_pool(name="const", bufs=1) as constp,
            tc.tile_pool(name="dram", bufs=1, space="DRAM") as dramp,
        ):
            ident = constp.tile([128, 128], f32)
            make_identity(nc, ident)
            cc_in = dramp.tile([3, 128, SHARD], f32)
            cc_out = dramp.tile([4, 3, 128, SHARD], f32)

            # ---------------- phase 1: routing + compress ----------------
            with (
                tc.tile_pool(name="p1c", bufs=1) as p1c,
                tc.tile_pool(name="p1w", bufs=2) as p1w,
                tc.tile_pool(name="p1s", bufs=4) as p1s,
                tc.tile_pool(name="p1ps", bufs=2, space="PSUM") as p1ps,
                tc.tile_pool(name="p1pw", bufs=2, space="PSUM") as p1pw,
            ):
                cfl = p1c.tile([128, 8, RANK * NCMP], f32)
                xt = p1c.tile([128, 8, SHARD], f32)
                rt = p1c.tile([128, 8, 48], f32)
                for dk in range(8):
                    nc.sync.dma_start(xt[:, dk, :], xT_d[dk])
                    nc.sync.dma_start(rt[:, dk, :], routersT_d[dk])
                    nc.sync.dma_start(cfl[:, dk, :], cflat_d[dk])

                for t in range(SHARD // 128):
                    # router scores for all 3 routers: [128, 48]
                    w3ps = p1pw.tile([128, 48], f32, tag="w3ps")
                    for dk in range(8):
                        nc.tensor.matmul(
                            w3ps[:],
                            xt[:, dk, ts(t, 128)],
                            rt[:, dk, :],
                            start=(dk == 0),
                            stop=(dk == 7),
                        )
                    w3 = p1s.tile([128, 48], f32, tag="w3")
                    nc.scalar.copy(w3[:], w3ps[:])
                    w3n = p1s.tile([128, 48], f32, tag="w3n")
                    for r in range(3):
                        sl = w3[:, ts(r, 16)]
                        negm = p1s.tile([128, 1], f32, tag="negm")
                        nc.vector.tensor_reduce(
                            negm[:], sl, axis=mybir.AxisListType.X,
                            op=mybir.AluOpType.max, negate=True,
                        )
                        e3 = p1s.tile([128, 16], f32, tag="e3")
                        z3 = p1s.tile([128, 1], f32, tag="z3")
                        nc.scalar.activation(
                            e3[:], sl, Exp, bias=negm[:], scale=1.0, accum_out=z3[:]
                        )
                        rz3 = p1s.tile([128, 1], f32, tag="rz3")
                        nc.vector.reciprocal(rz3[:], z3[:])
                        nc.vector.tensor_scalar_mul(w3n[:, ts(r, 16)], e3[:], rz3[:])

                    # y[s, (r, n)] = x @ C  (r outer, n inner in free dim)
                    y_sb = p1w.tile([128, RANK * NCMP], f32, tag="y")
                    for ch in range(4):
                        yps = p1ps.tile([128, 512], f32, tag="yps")
                        for dk in range(8):
                            nc.tensor.matmul(
                                yps[:],
                                xt[:, dk, ts(t, 128)],
                                cfl[:, dk, ts(ch, 512)],
                                start=(dk == 0),
                                stop=(dk == 7),
                            )
                        nc.scalar.copy(y_sb[:, ts(ch, 512)], yps[:])

                    yv = y_sb[:].rearrange("p (r n) -> p r n", n=NCMP)
                    for r in range(3):
                        tmp = p1w.tile([128, RANK * NCMP], f32, tag="tmp")
                        tv = tmp[:].rearrange("p (r n) -> p r n", n=NCMP)
                        for n in range(NCMP):
                            nc.vector.tensor_scalar_mul(
                                tv[:, :, n], yv[:, :, n],
                                w3n[:, r * 16 + n : r * 16 + n + 1],
                            )
                        h = p1s.tile([128, RANK], f32, tag="h")
                        nc.vector.tensor_reduce(
                            h[:], tv, axis=mybir.AxisListType.X, op=mybir.AluOpType.add
                        )
                        hT = p1ps.tile([128, 128], f32, tag="hT")
                        nc.tensor.transpose(hT[:], h[:], ident[:])
                        hTs = p1s.tile([128, 128], f32, tag="hTs")
                        nc.any.tensor_copy(hTs[:], hT[:])
                        nc.sync.dma_start(cc_in[r, :, ts(t, 128)], hTs[:])

            # ---------------- phase 2: allgather h^T within batch group ----
            nc.gpsimd.collective_compute(
                "AllGather",
                mybir.AluOpType.bypass,
                replica_groups=[[0, 1, 2, 3], [4, 5, 6, 7]],
                ins=[cc_in[:]],
                outs=[cc_out[:]],
            )

            # ---------------- phase 3: expand + attention + wO -------------
            with (
                tc.tile_pool(name="p3c", bufs=1) as p3c,
                tc.tile_pool(name="p3s", bufs=4) as p3s,
                tc.tile_pool(name="p3p", bufs=2) as p3p,
                tc.tile_pool(name="diagp", bufs=2) as diagp,
                tc.tile_pool(name="ptsp", bufs=2) as ptsp,
                tc.tile_pool(name="aop", bufs=2) as aop,
            ):
                hxt = p3c.tile([128, 3, S], f32)
                for r in range(3):
                    src = bass.AP(
                        tensor=cc_out.tensor,
                        offset=cc_out.offset + r * (128 * SHARD),
                        ap=[[SHARD, 128], [3 * 128 * SHARD, 4], [1, SHARD]],
                    )
                    nc.sync.dma_start(hxt[:, r, :], src)
                wq = p3c.tile([128, 3, HPC * DH], f32)
                for r in range(3):
                    nc.sync.dma_start(wq[:, r, :], wqkvT_d[r])
                wo = p3c.tile([128, 2, D], f32)
                for k in range(2):
                    nc.sync.dma_start(wo[:, k, :], wOT_d[k])
                if nt:
                    masks = p3c.tile([128, nt, CH], f32)
                    for t in range(nt):
                        nc.sync.dma_start(masks[:, t, :], dmask_d[t])

                QT = p3c.tile([128, 2, S], f32)
                KT = p3c.tile([128, 2, S], f32)
                V = p3c.tile([128, QT_TILES, HPC * DH], f32)
                with tc.tile_pool(name="qkps", bufs=3, space="PSUM") as qkps:
                    for r, dst in ((0, QT), (1, KT)):
                        for pair in range(2):
                            for ch in range(NCHUNK):
                                ps = qkps.tile([128, 512], f32, tag="qk")
                                nc.tensor.matmul(
                                    ps[:],
                                    wq[:, r, ts(pair, 128)],
                                    hxt[:, r, ts(ch, 512)],
                                    start=True,
                                    stop=True,
                                )
                                nc.scalar.copy(dst[:, pair, ts(ch, 512)], ps[:])
                    for st in range(QT_TILES):
                        vps = qkps.tile([128, HPC * DH], f32, tag="v")
                        nc.tensor.matmul(
                            vps[:],
                            hxt[:, 2, ts(st, 128)],
                            wq[:, 2, :],
                            start=True,
                            stop=True,
                        )
                        nc.vector.tensor_copy(V[:, st, :], vps[:])

                with (
                    tc.tile_pool(name="sps", bufs=1, space="PSUM") as spsp,
                    tc.tile_pool(name="ptp", bufs=2, space="PSUM") as ptp,
                    tc.tile_pool(name="avp", bufs=1, space="PSUM") as avp,
                    tc.tile_pool(name="op", bufs=1, space="PSUM") as opp,
                ):
                    for qi in range(QT_TILES):
                        stats, nch, nkt = plan[qi]
                        ncols = nch * CH
                        av = avp.tile([128, 2, 128], f32, tag="av")
                        for h in range(HPC):
                            pair, off = h // 2, (h % 2) * 64
                            sps = spsp.tile([128, S], f32, tag="s")
                            for c in range(nch):
                                if stats[c] == "off":
                                    nc.vector.memset(sps[:, ts(c, CH)], -1e30)
                                    continue
                                nc.tensor.matmul(
                                    sps[:, ts(c, CH)],
                                    QT[off : off + 64, pair, ts(qi, 128)],
                                    KT[off : off + 64, pair, ts(c, CH)],
                                    start=True,
                                    stop=True,
                                )
                                if stats[c] != "full":
                                    nc.vector.tensor_add(
                                        sps[:, ts(c, CH)],
                                        sps[:, ts(c, CH)],
                                        masks[:, stats[c], :],
                                    )
                            negm = p3s.tile([128, 1], f32, tag="negm")
                            nc.vector.tensor_reduce(
                                negm[:], sps[:, 0:ncols], axis=mybir.AxisListType.X,
                                op=mybir.AluOpType.max, negate=True,
                            )
                            p_sb = p3p.tile([128, S], f32, tag="p")
                            zz = p3s.tile([128, 1], f32, tag="z")
                            nc.scalar.activation(
                                p_sb[:, 0:ncols], sps[:, 0:ncols], Exp,
                                bias=negm[:], scale=1.0, accum_out=zz[:],
                            )
                            rz = p3s.tile([128, 1], f32, tag="rz")
                            nc.vector.reciprocal(rz[:], zz[:])
                            diag = diagp.tile([128, 128], f32, tag="diag")
                            nc.vector.tensor_scalar_mul(diag[:], ident[:], rz[:])
                            # scaled-transpose all attn tiles first, then run
                            # the AV accumulation as one uninterrupted group
                            pts = ptsp.tile([128, QT_TILES * 128], f32, tag="pts")
                            for kt in range(nkt):
                                ptps = ptp.tile([128, 128], f32, tag="pt")
                                nc.tensor.matmul(
                                    ptps[:], p_sb[:, ts(kt, 128)], diag[:],
                                    start=True, stop=True,
                                )
                                nc.any.tensor_copy(pts[:, ts(kt, 128)], ptps[:])
                            for kt in range(nkt):
                                nc.tensor.matmul(
                                    av[off : off + 64, pair, :],
                                    V[:, kt, ts(h, DH)],
                                    pts[:, ts(kt, 128)],
                                    start=(kt == 0),
                                    stop=(kt == nkt - 1),
                                    skip_group_check=True,
                                )
                        aoT = aop.tile([128, 2, 128], f32, tag="aoT")
                        for pair in range(2):
                            nc.any.tensor_copy(aoT[:, pair, :], av[:, pair, :])
                        for e in range(2):
                            ops = opp.tile([128, 512], f32, tag="o")
                            nc.tensor.matmul(
                                ops[:], aoT[:, 0, :], wo[:, 0, ts(e, 512)],
                                start=True, stop=False,
                            )
                            nc.tensor.matmul(
                                ops[:], aoT[:, 1, :], wo[:, 1, ts(e, 512)],
                                start=False, stop=True,
                            )
                            osb = p3p.tile([128, 512], f32, tag="osb")
                            nc.any.tensor_copy(osb[:], ops[:])
                            nc.sync.dma_start(out_d[ts(qi, 128), ts(e, 512)], osb[:])

    return nc


def _make_runner(plan, nt):
    """Compile the graph once and return fn(in_maps) -> list of out arrays."""
    import jax
    import numpy as np
    from jax.sharding import Mesh, PartitionSpec
    from jax.experimental.shard_map import shard_map
    import concourse.bass2jax as bass2jax
    import concourse.mybir as mybir

    nc = _build(plan, nt)
    bass2jax.install_neuronx_cc_hook()

    partition_name = nc.partition_id_tensor.name if nc.partition_id_tensor else None
    in_names, out_names, out_avals = [], [], []
    for alloc in nc.m.functions[0].allocations:
        if not isinstance(alloc, mybir.MemoryLocationSet):
            continue
        name = alloc.memorylocations[0].name
        if alloc.kind == "ExternalInput":
            if name != partition_name:
                in_names.append(name)
        elif alloc.kind == "ExternalOutput":
            out_names.append(name)
            out_avals.append(
                jax.core.ShapedArray(
                    tuple(alloc.tensor_shape), mybir.dt.np(alloc.dtype)
                )
            )
    n_params = len(in_names)
    all_names = in_names + out_names
    if partition_name is not None:
        all_names = all_names + [partition_name]

    def _body(*args):
        operands = list(args)
        if partition_name is not None:
            operands.append(bass2jax.partition_id_tensor())
        outs = bass2jax._bass_exec_p.bind(
            *operands,
            out_avals=tuple(out_avals),
            in_names=tuple(all_names),
            out_names=tuple(out_names),
            lowering_input_output_aliases=(),
            sim_require_finite=True,
            sim_require_nnan=True,
            nc=nc,
        )
        return tuple(outs)

    devices = jax.devices()[:N_CORES]
    mesh = Mesh(np.asarray(devices), ("core",))
    spec = (PartitionSpec("core"),)
    sharded = jax.jit(
        shard_map(
            _body,
            mesh=mesh,
            in_specs=spec * (n_params + len(out_names)),
            out_specs=spec * len(out_names),
            check_rep=False,
        ),
        keep_unused=True,
    )
    zeros = [
        np.zeros((N_CORES * a.shape[0], *a.shape[1:]), a.dtype) for a in out_avals
    ]

    def make_args(in_maps, device=False):
        concat = [
            np.concatenate([np.asarray(m[n]) for m in in_maps], axis=0)
            for n in in_names
        ] + list(zeros)
        if device:
            from jax.sharding import NamedSharding

            sh = NamedSharding(mesh, PartitionSpec("core"))
            concat = [jax.device_put(a, sh) for a in concat]
        return concat

    def run(in_maps):
        outs = sharded(*make_args(in_maps))
        res = np.asarray(outs[out_names.index("out")])
        return res.reshape(N_CORES, S, D)

    run.sharded = sharded
    run.make_args = make_args
    run.out_index = out_names.index("out")
    return run


def _prepare(inputs):
    """Host-side prep: mask plan + per-core input maps."""
    x = np.asarray(inputs["x"], np.float32)
    mask = np.asarray(inputs["mask"], bool)[0, 0]
    compress = np.asarray(inputs["compress_neurons"], np.float32)
    rQ = np.asarray(inputs["router_Q"], np.float32)
    rK = np.asarray(inputs["router_K"], np.float32)
    rV = np.asarray(inputs["router_V"], np.float32)
    wQ = np.asarray(inputs["wQ"], np.float32)
    wK = np.asarray(inputs["wK"], np.float32)
    wV = np.asarray(inputs["wV"], np.float32)
    wO = np.asarray(inputs["wO"], np.float32)

    plan, mtiles = _mask_plan(mask)
    nt = len(mtiles)

    # host-side shared prep
    cflat = np.ascontiguousarray(
        compress.transpose(1, 2, 0).reshape(8, 128, RANK * NCMP)
    )  # [D, R, NC] -> d-tiles
    routersT = np.ascontiguousarray(
        np.stack([rQ, rK, rV]).transpose(2, 0, 1).reshape(8, 128, 48)
    )
    wqT = wQ.T * np.float32(1.0 / np.sqrt(DH))  # fold 1/sqrt(dh) into Q
    wkT, wvT = wK.T, wV.T
    wOT = np.ascontiguousarray(wO.T)  # [D, E]

    in_maps = []
    for c in range(N_CORES):
        b, q4 = divmod(c, 4)
        hs = slice(HPC * q4 * DH, HPC * q4 * DH + HPC * DH)
        m = {
            "xT": np.ascontiguousarray(x[b, q4 * SHARD : (q4 + 1) * SHARD, :].T)
            .reshape(8, 128, SHARD),
            "cflat": cflat,
            "routersT": routersT,
            "wqkvT": np.ascontiguousarray(
                np.stack([wqT[:, hs], wkT[:, hs], wvT[:, hs]])
            ),
            "wOT": np.ascontiguousarray(wOT[hs, :]).reshape(2, 128, D),
        }
        if nt:
            m["dmask"] = mtiles
        in_maps.append(m)
    return plan, nt, in_maps


def kernel(**inputs):
    plan, nt, in_maps = _prepare(inputs)
    key = (plan, nt)
    if key not in _RUNNERS:
        _RUNNERS[key] = _make_runner(plan, nt)
    res = _RUNNERS[key](in_maps)  # [8, S, D] partials
    out = np.empty((B, S, D), np.float32)
    for b in range(B):
        out[b] = res[4 * b : 4 * b + 4].sum(axis=0, dtype=np.float64)
    return out


# revision 34
# speedup vs baseline: 32.2620x; 32.2620x over previous
"""MoE-routed low-rank attention (nn_NeuronCircuit_28930899706061) on 8 TRN2 cores.

Sharding: core c in 0..7 -> batch b = c//4, token-shard q4 = c%4 (512 tokens)
for the compress/routing phase; head group heads [4*q4, 4*q4+4) of batch b for
the attention phase. h^T tensors are all-gathered within each 4-core batch
group; each core emits a partial [S, D] output (its 4 heads' contribution
through wO) and the host sums the 4 partials per batch.

All weight transposes are done host-side so every device DMA is contiguous.
"""

import numpy as np

B, S, D, H, RANK, NCMP = 2, 2048, 1024, 16, 128, 16
DH = D // H  # 64
N_CORES = 8
SHARD = S // 4  # 512 tokens per core in phase 1
HPC = 4  # heads per core
QT_TILES = S // 128  # 16 q tiles
CH = 512  # k-chunk width (one PSUM bank)
NCHUNK = S // CH  # 4

_RUNNERS: dict = {}


def _split_multi_waits(nc, mybir):
    """This toolchain's walrus rejects any instruction carrying >1 sync wait
    ("Too many sync wait commands"); hoist excess waits onto same-engine nops
    inserted immediately before the instruction."""
    cnt = 0
    for f in nc.m.functions:
        for blk in f.blocks:
            il = blk.instructions
            out = []
            changed = False
            for inst in il:
                si = inst.sync_info
                waits = list(si.on_wait or []) if si else []
                if len(waits) > 1:
                    for w in waits[:-1]:
                        cnt += 1
                        nop = mybir.InstNoOp(
                            name=f"wsplit-{cnt}",
                            engine=inst.engine,
                            sync_info=mybir.SyncInfo(on_wait=[w], on_update=[]),
                        )
                        nc.register_instruction(nop)
                        out.append(nop)
                    inst.sync_info = mybir.SyncInfo(
                        on_wait=[waits[-1]], on_update=list(si.on_update or [])
                    )
                    changed = True
                out.append(inst)
            if changed:
                il[:] = out


def _make_tc_class(tile, mybir):
    class TC(tile.TileContext):
        def __exit__(self, *exc):
            ret = super().__exit__(*exc)
            if exc[0] is None:
                _split_multi_waits(self.nc, mybir)
            return ret

    return TC


def _mask_plan(maskb):
    """Per q-tile chunk statuses + true k-tile count, plus packed mask tiles.

    Returns (plan, tiles) where plan[qi] = (chunk_stats, nch, nkt) with
    chunk_stats a list of ('full'|'off'|int mask-tile index) for chunks
    0..nch-1, and tiles a [nt,128,512] f32 array of additive masks.
    """
    tiles = []
    tile_ids = {}
    plan = []
    for qi in range(QT_TILES):
        rows = maskb[qi * 128 : (qi + 1) * 128]
        stats = []
        for c in range(NCHUNK):
            sub = rows[:, c * CH : (c + 1) * CH]
            if sub.all():
                stats.append("full")
            elif not sub.any():
                stats.append("off")
            else:
                add = np.where(sub, np.float32(0), np.float32(-1e30))
                key = add.tobytes()
                if key not in tile_ids:
                    tile_ids[key] = len(tiles)
                    tiles.append(add)
                stats.append(tile_ids[key])
        # trim trailing 'off' chunks
        nch = NCHUNK
        while nch > 0 and stats[nch - 1] == "off":
            nch -= 1
        # true k-tiles (128 wide) with any active column
        nkt = 0
        for kt in range(S // 128):
            if rows[:, kt * 128 : (kt + 1) * 128].any():
                nkt = kt + 1
        plan.append((tuple(stats[:nch]), nch, nkt))
    nt = len(tiles)
    tiles_arr = (
        np.stack(tiles).astype(np.float32) if nt else np.zeros((0, 128, CH), np.float32)
    )
    return tuple(plan), tiles_arr


def _bcast_mid(bass, ap, n):
    """[P, K] AP -> [P, n, K] AP with a step-0 middle dim (free broadcast)."""
    dims = [list(x) for x in ap.ap]
    return bass.AP(
        tensor=ap.tensor, offset=ap.offset, ap=[dims[0], [0, n]] + dims[1:]
    )


def _build(plan, nt, repeat=1, skip_cc=False, skip_p1=False):
    import concourse.bass as bass
    import concourse.mybir as mybir
    import concourse.tile as tile
    from concourse.bass import ts
    from concourse.masks import make_identity

    f32 = mybir.dt.float32
    bf16 = mybir.dt.bfloat16
    Exp = mybir.ActivationFunctionType.Exp
    TC = _make_tc_class(tile, mybir)

    nc = bass.Bass(num_devices=N_CORES)
    xT_d = nc.dram_tensor("xT", [8, 128, SHARD], bf16, kind="ExternalInput")
    cflat_d = nc.dram_tensor("cflat", [8, 128, RANK * NCMP], bf16, kind="ExternalInput")
    routersT_d = nc.dram_tensor("routersT", [8, 128, 48], bf16, kind="ExternalInput")
    wqkvT_d = nc.dram_tensor("wqkvT", [3, 128, HPC * DH], bf16, kind="ExternalInput")
    wOT_d = nc.dram_tensor("wOT", [2, 128, D], bf16, kind="ExternalInput")
    dmask_d = (
        nc.dram_tensor("dmask", [nt, 128, CH], f32, kind="ExternalInput")
        if nt
        else None
    )
    out_d = nc.dram_tensor("out", [S, D], bf16, kind="ExternalOutput")

    groups = [[0, 1, 2, 3], [4, 5, 6, 7]]
    NT4 = SHARD // 128  # 4 s-tiles per core in phase 1

    with TC(nc) as tc:
      for _rep in range(repeat):
        # one flat SBUF region for both phases -- everything fits, and this
        # avoids a pool-release barrier between compress and attention
        with (
            tc.tile_pool(name="sb", bufs=1) as sbp,
            tc.tile_pool(name="wk2", bufs=2) as wk2,
            tc.tile_pool(name="wk4", bufs=4) as wk4,
            tc.tile_pool(name="dram", bufs=1, space="DRAM") as dramp,
        ):
            ident = sbp.tile([128, 128], f32)
            make_identity(nc, ident)
            ident16 = sbp.tile([128, 128], bf16)
            make_identity(nc, ident16)
            cbias = sbp.tile([128, 1], f32)
            nc.vector.memset(cbias[:], -20.0)
            cc_in = [
                dramp.tile([128, SHARD], bf16, name=f"cc_in{r}") for r in range(3)
            ]
            cc_out = [
                dramp.tile([4, 128, SHARD], bf16, name=f"cc_out{r}")
                for r in range(3)
            ]


            # attention-side constant loads (no deps -- schedule early)
            wq = sbp.tile([128, 3, HPC * DH], bf16)
            for r in range(3):
                nc.sync.dma_start(wq[:, r, :], wqkvT_d[r])
            wo = sbp.tile([128, 2, D], bf16)
            for k in range(2):
                nc.sync.dma_start(wo[:, k, :], wOT_d[k])
            if nt:
                masks = sbp.tile([128, nt, CH], f32)
                for t in range(nt):
                    nc.sync.dma_start(masks[:, t, :], dmask_d[t])

            with (
                tc.tile_pool(name="p1ps", bufs=2, space="PSUM") as p1ps,
                tc.tile_pool(name="p1pw", bufs=1, space="PSUM") as p1pw,
                tc.tile_pool(name="p1pt", bufs=2, space="PSUM") as p1pt,
                tc.tile_pool(name="qkps", bufs=3, space="PSUM") as qkps,
            ):
                # ------------- phase 1: routing + compress -------------
                if skip_p1:
                    for r in range(3):
                        nc.gpsimd.collective_compute(
                            "AllGather", mybir.AluOpType.bypass,
                            replica_groups=groups,
                            ins=[cc_in[r][:]], outs=[cc_out[r][:]],
                        )
                else:
                    cfl = sbp.tile([128, 8, RANK * NCMP], bf16)
                    xt = sbp.tile([128, 8, SHARD], bf16)
                    rt = sbp.tile([128, 8, 48], bf16)
                    for dk in range(8):
                        nc.sync.dma_start(xt[:, dk, :], xT_d[dk])
                        nc.sync.dma_start(rt[:, dk, :], routersT_d[dk])
                        nc.sync.dma_start(cfl[:, dk, :], cflat_d[dk])

                    ys = [
                        sbp.tile([128, RANK * NCMP], bf16, name=f"y{t}")
                        for t in range(NT4)
                    ]
                    w3ns = [
                        sbp.tile([128, 48], f32, name=f"w3n{t}") for t in range(NT4)
                    ]
                    for t in range(NT4):
                        w3ps = p1pw.tile([128, 48], f32, tag="w3ps")
                        for dk in range(8):
                            nc.tensor.matmul(
                                w3ps[:], xt[:, dk, ts(t, 128)], rt[:, dk, :],
                                start=(dk == 0), stop=(dk == 7),
                            )
                        w3 = wk4.tile([128, 48], f32, tag="w3")
                        nc.scalar.copy(w3[:], w3ps[:])
                        w3n = w3ns[t]
                        for r in range(3):
                            # no max-subtraction: router logits are O(1)
                            e3 = wk4.tile([128, 16], f32, tag="e3")
                            z3 = wk4.tile([128, 1], f32, tag="z3")
                            nc.scalar.activation(
                                e3[:], w3[:, ts(r, 16)], Exp,
                                bias=0.0, scale=1.0, accum_out=z3[:],
                            )
                            rz3 = wk4.tile([128, 1], f32, tag="rz3")
                            nc.vector.reciprocal(rz3[:], z3[:])
                            nc.vector.tensor_scalar_mul(
                                w3n[:, ts(r, 16)], e3[:], rz3[:]
                            )

                        # y[s, (r, n)] = x @ C  (r outer, n inner)
                        for ch in range(4):
                            yps = p1ps.tile([128, 512], f32, tag="yps")
                            for dk in range(8):
                                nc.tensor.matmul(
                                    yps[:], xt[:, dk, ts(t, 128)],
                                    cfl[:, dk, ts(ch, 512)],
                                    start=(dk == 0), stop=(dk == 7),
                                )
                            nc.any.tensor_copy(ys[t][:, ts(ch, 512)], yps[:])

                    # per router: combine, transpose, gather -- router r's
                    # collective overlaps router r+1's combines
                    for r in range(3):
                        for t in range(NT4):
                            yv = ys[t][:].rearrange("p (r n) -> p r n", n=NCMP)
                            tmp = wk2.tile([128, RANK * NCMP], bf16, tag="tmp")
                            tv = tmp[:].rearrange("p (r n) -> p r n", n=NCMP)
                            wb = _bcast_mid(bass, w3ns[t][:, ts(r, 16)], RANK)
                            nc.vector.tensor_mul(tv, yv, wb)
                            h = wk4.tile([128, RANK], f32, tag="h")
                            nc.vector.tensor_reduce(
                                h[:], tv, axis=mybir.AxisListType.X,
                                op=mybir.AluOpType.add,
                            )
                            hT = p1pt.tile([128, 128], f32, tag="hT")
                            nc.tensor.transpose(hT[:], h[:], ident[:])
                            hTs = wk4.tile([128, 128], bf16, tag="hTs")
                            nc.any.tensor_copy(hTs[:], hT[:])
                            nc.sync.dma_start(cc_in[r][:, ts(t, 128)], hTs[:])
                        if not skip_cc:
                            nc.gpsimd.collective_compute(
                                "AllGather", mybir.AluOpType.bypass,
                                replica_groups=groups,
                                ins=[cc_in[r][:]], outs=[cc_out[r][:]],
                            )
                        else:
                            for g in range(4):
                                nc.sync.dma_start(cc_out[r][g], cc_in[r][:])

                # ------------- expand: Q^T, K^T, V -------------
                hxt = sbp.tile([128, 3, S], bf16)
                for r in range(3):
                    srcap = bass.AP(
                        tensor=cc_out[r].tensor,
                        offset=cc_out[r].offset,
                        ap=[[SHARD, 128], [128 * SHARD, 4], [1, SHARD]],
                    )
                    nc.sync.dma_start(hxt[:, r, :], srcap)

                QT = sbp.tile([128, 2, S], bf16)
                KT = sbp.tile([128, 2, S], bf16)
                V = sbp.tile([128, QT_TILES, HPC * DH], bf16)
                for r, dst in ((0, QT), (1, KT)):
                    for pair in range(2):
                        for ch in range(NCHUNK):
                            ps = qkps.tile([128, 512], f32, tag="qk")
                            nc.tensor.matmul(
                                ps[:], wq[:, r, ts(pair, 128)],
                                hxt[:, r, ts(ch, 512)],
                                start=True, stop=True,
                            )
                            nc.any.tensor_copy(dst[:, pair, ts(ch, 512)], ps[:])
                for st in range(QT_TILES):
                    vps = qkps.tile([128, HPC * DH], f32, tag="qk")
                    nc.tensor.matmul(
                        vps[:], hxt[:, 2, ts(st, 128)], wq[:, 2, :],
                        start=True, stop=True,
                    )
                    nc.vector.tensor_copy(V[:, st, :], vps[:])

            # ------------- attention + wO -------------
            with (
                tc.tile_pool(name="sps", bufs=3, space="PSUM") as spsp,
                tc.tile_pool(name="ptp", bufs=2, space="PSUM") as ptp,
                tc.tile_pool(name="avp", bufs=1, space="PSUM") as avp,
                tc.tile_pool(name="op", bufs=2, space="PSUM") as opp,
            ):
                for qi in range(QT_TILES):
                    stats, nch, nkt = plan[qi]
                    av = avp.tile([128, 2, 128], f32, tag="av")
                    for h in range(HPC):
                        pair, off = h // 2, (h % 2) * 64
                        # exp(score - 20) per chunk; the constant shift
                        # cancels exactly in the softmax normalization
                        p_sb = wk2.tile([128, S], bf16, tag="p")
                        zp = wk4.tile([128, 4], f32, tag="zp")
                        for c in range(nch):
                            if stats[c] == "off":
                                nc.vector.memset(p_sb[:, ts(c, CH)], 0.0)
                                nc.vector.memset(zp[:, c : c + 1], 0.0)
                                continue
                            sps = spsp.tile([128, CH], f32, tag="s")
                            nc.tensor.matmul(
                                sps[:],
                                QT[off : off + 64, pair, ts(qi, 128)],
                                KT[off : off + 64, pair, ts(c, CH)],
                                start=True, stop=True,
                            )
                            if stats[c] != "full":
                                nc.vector.tensor_add(
                                    sps[:], sps[:], masks[:, stats[c], :]
                                )
                            nc.scalar.activation(
                                p_sb[:, ts(c, CH)], sps[:], Exp,
                                bias=cbias[:], scale=1.0,
                                accum_out=zp[:, c : c + 1],
                            )
                        zz = wk4.tile([128, 1], f32, tag="z")
                        nc.vector.tensor_reduce(
                            zz[:], zp[:, 0:nch], axis=mybir.AxisListType.X,
                            op=mybir.AluOpType.add,
                        )
                        rz = wk4.tile([128, 1], f32, tag="rz")
                        nc.vector.reciprocal(rz[:], zz[:])
                        diag = wk2.tile([128, 128], bf16, tag="diag")
                        nc.vector.tensor_scalar_mul(diag[:], ident16[:], rz[:])
                        # scaled-transpose all attn tiles, then one
                        # uninterrupted AV accumulation group
                        pts = wk2.tile([128, QT_TILES * 128], bf16, tag="pts")
                        for kb in range(0, nkt, 4):
                            kw = min(4, nkt - kb)
                            ptps = ptp.tile([128, 512], f32, tag="pt")
                            for j in range(kw):
                                nc.tensor.matmul(
                                    ptps[:, ts(j, 128)],
                                    p_sb[:, ts(kb + j, 128)], diag[:],
                                    start=True, stop=True,
                                    skip_group_check=True,
                                )
                            nc.any.tensor_copy(
                                pts[:, kb * 128 : (kb + kw) * 128],
                                ptps[:, 0 : kw * 128],
                            )
                        for kt in range(nkt):
                            nc.tensor.matmul(
                                av[off : off + 64, pair, :],
                                V[:, kt, ts(h, DH)],
                                pts[:, ts(kt, 128)],
                                start=(kt == 0), stop=(kt == nkt - 1),
                                skip_group_check=True,
                            )
                    aoT = wk2.tile([128, 2, 128], bf16, tag="aoT")
                    for pair in range(2):
                        nc.any.tensor_copy(aoT[:, pair, :], av[:, pair, :])
                    for e in range(2):
                        ops = opp.tile([128, 512], f32, tag="o")
                        nc.tensor.matmul(
                            ops[:], aoT[:, 0, :], wo[:, 0, ts(e, 512)],
                            start=True, stop=False,
                        )
                        nc.tensor.matmul(
                            ops[:], aoT[:, 1, :], wo[:, 1, ts(e, 512)],
                            start=False, stop=True,
                        )
                        osb = wk2.tile([128, 512], bf16, tag="osb")
                        nc.any.tensor_copy(osb[:], ops[:])
                        nc.sync.dma_start(out_d[ts(qi, 128), ts(e, 512)], osb[:])

    return nc


def _make_runner(plan, nt, repeat=1):
    """Compile the graph once and return fn(in_maps) -> list of out arrays."""
    import jax
    import numpy as np
    from jax.sharding import Mesh, PartitionSpec
    from jax.experimental.shard_map import shard_map
    import concourse.bass2jax as bass2jax
    import concourse.mybir as mybir

    nc = _build(plan, nt, repeat=repeat)
    bass2jax.install_neuronx_cc_hook()

    partition_name = nc.partition_id_tensor.name if nc.partition_id_tensor else None
    in_names, out_names, out_avals = [], [], []
    for alloc in nc.m.functions[0].allocations:
        if not isinstance(alloc, mybir.MemoryLocationSet):
            continue
        name = alloc.memorylocations[0].name
        if alloc.kind == "ExternalInput":
            if name != partition_name:
                in_names.append(name)
        elif alloc.kind == "ExternalOutput":
            out_names.append(name)
            out_avals.append(
                jax.core.ShapedArray(
                    tuple(alloc.tensor_shape), mybir.dt.np(alloc.dtype)
                )
            )
    n_params = len(in_names)
    all_names = in_names + out_names
    if partition_name is not None:
        all_names = all_names + [partition_name]

    def _body(*args):
        operands = list(args)
        if partition_name is not None:
            operands.append(bass2jax.partition_id_tensor())
        outs = bass2jax._bass_exec_p.bind(
            *operands,
            out_avals=tuple(out_avals),
            in_names=tuple(all_names),
            out_names=tuple(out_names),
            lowering_input_output_aliases=(),
            sim_require_finite=True,
            sim_require_nnan=True,
            nc=nc,
        )
        return tuple(outs)

    devices = jax.devices()[:N_CORES]
    mesh = Mesh(np.asarray(devices), ("core",))
    SHARED = {"cflat", "routersT", "dmask"}
    in_specs = tuple(
        PartitionSpec() if n in SHARED else PartitionSpec("core") for n in in_names
    ) + (PartitionSpec("core"),) * len(out_names)
    sharded = jax.jit(
        shard_map(
            _body,
            mesh=mesh,
            in_specs=in_specs,
            out_specs=(PartitionSpec("core"),) * len(out_names),
            check_rep=False,
        ),
        keep_unused=True,
    )
    zeros = [
        np.zeros((N_CORES * a.shape[0], *a.shape[1:]), a.dtype) for a in out_avals
    ]

    def make_args(in_maps, device=False):
        arrs = []
        for n in in_names:
            if n in SHARED:
                arrs.append(np.asarray(in_maps[0][n]))
            else:
                arrs.append(
                    np.concatenate([np.asarray(m[n]) for m in in_maps], axis=0)
                )
        arrs += list(zeros)
        if device:
            from jax.sharding import NamedSharding

            for i, n in enumerate(in_names):
                sh = NamedSharding(
                    mesh, PartitionSpec() if n in SHARED else PartitionSpec("core")
                )
                arrs[i] = jax.device_put(arrs[i], sh)
            sh = NamedSharding(mesh, PartitionSpec("core"))
            for i in range(len(in_names), len(arrs)):
                arrs[i] = jax.device_put(arrs[i], sh)
        return arrs

    def run(in_maps):
        outs = sharded(*make_args(in_maps))
        res = np.asarray(outs[out_names.index("out")])
        return res.reshape(N_CORES, S, D)

    run.sharded = sharded
    run.make_args = make_args
    run.out_index = out_names.index("out")
    return run


def _prepare(inputs):
    """Host-side prep: mask plan + per-core input maps."""
    x = np.asarray(inputs["x"], np.float32)
    mask = np.asarray(inputs["mask"], bool)[0, 0]
    compress = np.asarray(inputs["compress_neurons"], np.float32)
    rQ = np.asarray(inputs["router_Q"], np.float32)
    rK = np.asarray(inputs["router_K"], np.float32)
    rV = np.asarray(inputs["router_V"], np.float32)
    wQ = np.asarray(inputs["wQ"], np.float32)
    wK = np.asarray(inputs["wK"], np.float32)
    wV = np.asarray(inputs["wV"], np.float32)
    wO = np.asarray(inputs["wO"], np.float32)

    plan, mtiles = _mask_plan(mask)
    nt = len(mtiles)

    # host-side shared prep
    import ml_dtypes

    bf = ml_dtypes.bfloat16
    cflat = np.ascontiguousarray(
        compress.transpose(1, 2, 0).reshape(8, 128, RANK * NCMP)
    ).astype(bf)  # [D, R, NC] -> d-tiles
    routersT = np.ascontiguousarray(
        np.stack([rQ, rK, rV]).transpose(2, 0, 1).reshape(8, 128, 48)
    ).astype(bf)
    wqT = wQ.T * np.float32(1.0 / np.sqrt(DH))  # fold 1/sqrt(dh) into Q
    wkT, wvT = wK.T, wV.T
    wOT = np.ascontiguousarray(wO.T).astype(bf)  # [D, E]

    in_maps = []
    for c in range(N_CORES):
        b, q4 = divmod(c, 4)
        hs = slice(HPC * q4 * DH, HPC * q4 * DH + HPC * DH)
        m = {
            "xT": np.ascontiguousarray(x[b, q4 * SHARD : (q4 + 1) * SHARD, :].T)
            .reshape(8, 128, SHARD)
            .astype(bf),
            "cflat": cflat,
            "routersT": routersT,
            "wqkvT": np.ascontiguousarray(
                np.stack([wqT[:, hs], wkT[:, hs], wvT[:, hs]])
            ),
            "wOT": np.ascontiguousarray(wOT[hs, :]).reshape(2, 128, D),
        }
        if nt:
            m["dmask"] = mtiles
        in_maps.append(m)
    return plan, nt, in_maps


def kernel(**inputs):
    plan, nt, in_maps = _prepare(inputs)
    key = (plan, nt)
    if key not in _RUNNERS:
        _RUNNERS[key] = _make_runner(plan, nt)
    res = _RUNNERS[key](in_maps)  # [8, S, D] bf16 partials
    out = np.empty((B, S, D), np.float32)
    for b in range(B):
        out[b] = res[4 * b : 4 * b + 4].astype(np.float32).sum(axis=0)
    return out
